# revision 13
# baseline (speedup 1.0000x reference)
"""ARB loss kernel for Trainium2, SPMD across 8 NeuronCores.

Reference computation (n=8192 rows, C=32000 classes):
    counts = bincount(y, C)                       # label histogram
    w[i]   = counts[y[i]]
    rowsum[i] = sum_c output[i, c]
    denom[i]  = (n / w[i]) * rowsum[i]
    loss = -mean_i log(output[i, y[i]] / denom[i])
         = log(n) - (1/n) * sum_i log(output[i,y[i]] * w[i] / rowsum[i])

The loss is scale-invariant in `output`, and the 2e-2 rel-err budget is
~500x looser than fp8 quantization error on this computation (measured
4.4e-5), so the stream is uploaded as fp8e4m3 (output * 64, exact power
of two so only the exponent shifts) - 4x less HBM traffic than f32.

Sharding: data-parallel over rows, 1024 rows per core. The row-sum
reduction is split across engines BY ROW so per-row partial sums never
need a cross-layout combine:
  - rows 0..639: uploaded TRANSPOSED; the Tensor engine row-sums them
    with fp8 DoubleRow matmuls against an all-ones weight vector
    (contraction = 256 columns per pass), accumulating into PSUM
    [1, 512] + [1, 128].
  - rows 640..1023: uploaded in row-major layout; the Scalar engine
    row-sums them with activation(Copy) + fused free-dim accumulation.
  - w (label counts) for all rows: labels as int16, replicated to all
    128 partitions via int16 PE matmuls against ones (exact for
    y < 2^15), PSUM evacuated by the Vector engine, which then runs
    tensor_scalar(is_equal) with fused add-reduction per 128-row block.
  - output[i, y[i]]: elementwise indirect DMA (1 byte) from a single
    flat dram tensor holding both layouts.
  - log() terms via Scalar engine Ln with fused accumulation.
Host unshard: loss = log(n) - (sum(acc_pos) - sum(acc_neg))/n.
"""

import math
import sys
from contextlib import ExitStack

import numpy as np

if "/opt/trn_rl_repo" not in sys.path:
    sys.path.insert(0, "/opt/trn_rl_repo")

# bass_utils imports antenv.axon_hooks when BASS_TRACE is set; make sure a
# stub exists so a missing module never crashes the run (trace then simply
# degrades to no-profile).
try:
    import antenv.axon_hooks  # noqa: F401
except ImportError:
    import types

    try:
        import antenv

        _stub = types.ModuleType("antenv.axon_hooks")
        _stub._HOOK = None
        _stub.set_axon_ntff_profile_hook = lambda h: setattr(_stub, "_HOOK", h)
        _stub.get_axon_ntff_profile_hook = lambda: _stub._HOOK
        sys.modules["antenv.axon_hooks"] = _stub
        antenv.axon_hooks = _stub
    except ImportError:
        pass

N = 8192           # total rows
C = 32000          # classes
NCORES = 8
RPC = N // NCORES  # rows per core = 1024
P = 128            # partitions
RB = RPC // P      # row blocks per core = 8
SCALE = 64.0       # fp8 pre-scale (power of two -> exponent shift only)

RT = 640           # rows reduced on the Tensor engine (transposed layout)
RTA = 512          # psum bank A rows
RTB = RT - RTA     # psum bank B rows = 128
RN = RPC - RT      # rows reduced on the Scalar engine = 384
NBLK = RN // P     # scalar-engine row blocks = 3
NCH = 2            # column chunks per scalar-engine row block
CHW = C // NCH     # chunk width = 16000

DC = C // 256      # DoubleRow chunks (256 cols each) = 125
G = 5              # DoubleRow chunks per full streamed tile
NFT = 24           # full xt tiles (G chunks each)
NST = DC - NFT * G # single-chunk tail tiles = 5
NTT = NFT + NST    # xt tiles = 29
FTB = G * 2 * RT   # bytes/partition/full tile = 6400
STB = 2 * RT       # bytes/partition/single tile = 1280
XT_BYTES = DC * 256 * RT       # 20480000
XN_BYTES = RN * C              # 12288000
TOTB = XT_BYTES + XN_BYTES     # 32768000

NBUF = 4           # xt stream buffers
NNBUF = 3          # xn stream buffers
NREP = N // 512    # label-replicate matmuls = 16
HALF = N // 2      # labels per count half-op
GATHER_AT = 12     # start indirect gathers after this many xt tiles

_CACHE = {}

# xt tile table: (dchunk0, n_dchunks, byte_base)
def _xt_tiles():
    tiles = []
    base = 0
    for t in range(NFT):
        tiles.append((t * G, G, base))
        base += P * FTB
    for s in range(NST):
        tiles.append((NFT * G + s, 1, base))
        base += P * STB
    assert base == XT_BYTES
    return tiles


# stream emission order: xn chunk j goes after the (4j)th xt tile so the
# scalar engine is fed at its consumption cadence; xt singles last
def _stream_order():
    order = []
    xn = 0
    for i in range(NTT):
        if i % 4 == 0 and xn < NBLK * NCH:
            order.append(("xn", xn))
            xn += 1
        order.append(("xt", i))
    while xn < NBLK * NCH:
        order.append(("xn", xn))
        xn += 1
    return order


def _build_nc():
    import concourse.bass as bass
    import concourse.mybir as mybir

    f32 = mybir.dt.float32
    i32 = mybir.dt.int32
    i16 = mybir.dt.int16
    bf16 = mybir.dt.bfloat16
    f8 = mybir.dt.float8e4

    nc = bass.Bass()
    xq_ext = nc.dram_tensor("xq", [1, TOTB], f8, kind="ExternalInput")
    yf_ext = nc.dram_tensor("yf", [1, N], i16, kind="ExternalInput")
    ylf_ext = nc.dram_tensor("ylf", [P, RB], f32, kind="ExternalInput")
    off_ext = nc.dram_tensor("off", [P, RB], i32, kind="ExternalInput")
    # DoubleRow LDWEIGHTS needs the two k-tile weights 16B apart
    # (s3_lw_dual_fp8_restrictions: num_elem[2]==2, step%16==0)
    w1_ext = nc.dram_tensor("w1", [P, 32], f8, kind="ExternalInput")
    out_ext = nc.dram_tensor("out", [P, 4], f32, kind="ExternalOutput")

    xt_tiles = _xt_tiles()
    order = _stream_order()

    with ExitStack() as es:
        ec = es.enter_context
        data = [
            ec(nc.sbuf_tensor(f"data{j}", [P, FTB], f8))
            for j in range(NBUF)
        ]
        ndata = [
            ec(nc.sbuf_tensor(f"ndata{j}", [P, CHW], f8))
            for j in range(NNBUF)
        ]
        yfb = ec(nc.sbuf_tensor([P, N], i16))
        w1_sb = ec(nc.sbuf_tensor([P, 32], f8))
        eqscr = ec(nc.sbuf_tensor([P, HALF], bf16))
        w_half = ec(nc.sbuf_tensor([P, 2 * RB], f32))
        w_sb = ec(nc.sbuf_tensor([P, RB], f32))
        tv8 = ec(nc.sbuf_tensor([P, RB], f8))
        tvf = ec(nc.sbuf_tensor([P, RB], f32))
        ylf_sb = ec(nc.sbuf_tensor([P, RB], f32))
        off_sb = ec(nc.sbuf_tensor([P, RB], i32))
        tprod = ec(nc.sbuf_tensor([P, RB], f32))
        logt = ec(nc.sbuf_tensor([P, RB], f32))
        logr = ec(nc.sbuf_tensor([1, RTA], f32))
        act_scr = ec(nc.sbuf_tensor([P, CHW], f8))
        act_part = ec(nc.sbuf_tensor([P, NBLK * NCH], f32))
        rs_n = ec(nc.sbuf_tensor([P, NBLK], f32))
        logn_scr = ec(nc.sbuf_tensor([P, NBLK], f32))
        acc = ec(nc.sbuf_tensor([P, 4], f32))

        rsA = ec(nc.psum_tensor("rsA", [P, 512], f32))
        rsB = ec(nc.psum_tensor("rsB", [P, 512], f32))
        dmaT = [ec(nc.semaphore(f"dmaT{j}")) for j in range(NBUF)]
        dmaN = [ec(nc.semaphore(f"dmaN{j}")) for j in range(NNBUF)]
        tsem = ec(nc.semaphore("tsem"))   # PE: +1 per finished xt tile
        bsem = ec(nc.semaphore("bsem"))   # label broadcast chunks
        nsem = ec(nc.semaphore("nsem"))   # ACT: +1 per xn chunk reduced
        dmaP = ec(nc.semaphore("dmaP"))
        dmaG = ec(nc.semaphore("dmaG"))
        vsem = ec(nc.semaphore("vsem"))
        asem = ec(nc.semaphore("asem"))
        block = ec(nc.Block())

        xq_flat = xq_ext[0:1, :].rearrange("a b -> (a b)")
        xn_rows = xq_ext[0:1, XT_BYTES:].rearrange("a (r c) -> (a r) c", c=C)

        @block.sync
        def _(sync):
            sync.dma_start(ylf_sb[:, :], ylf_ext[:, :]).then_inc(dmaP, 16)
            sync.dma_start(off_sb[:, :], off_ext[:, :]).then_inc(dmaP, 16)
            sync.dma_start(w1_sb[:, :], w1_ext[:, :]).then_inc(dmaP, 16)
            for kind, i in order:
                if kind == "xt":
                    buf = i % NBUF
                    if i >= NBUF:
                        sync.wait_ge(tsem, i - NBUF + 1)
                    _, g, base = xt_tiles[i]
                    w = g * STB
                    sync.dma_start(
                        data[buf][:, 0:w],
                        xq_flat[base : base + P * w].rearrange(
                            "(p f) -> p f", p=P
                        ),
                    ).then_inc(dmaT[buf], 16)
                else:
                    buf = i % NNBUF
                    if i >= NNBUF:
                        sync.wait_ge(nsem, i - NNBUF + 1)
                    b, k = divmod(i, NCH)
                    sync.dma_start(
                        ndata[buf][:, :],
                        xn_rows[b * P : (b + 1) * P, k * CHW : (k + 1) * CHW],
                    ).then_inc(dmaN[buf], 16)
            sync.wait_ge(asem, 5)
            sync.dma_start(out_ext[:, :], acc[:, :]).then_inc(dmaG, 16)

        @block.gpsimd
        def _(gpsimd):
            # replicate the label vector to all 128 partitions with
            # broadcast DMAs (SBUF->SBUF, own queue, no HBM traffic)
            gpsimd.wait_ge(dmaP, 48)
            for q in range(4):
                lo = q * (N // 4)
                hi = lo + N // 4
                gpsimd.dma_start(
                    yfb[:, lo:hi],
                    yf_ext[0:1, lo:hi].to_broadcast((P, N // 4)),
                ).then_inc(bsem, 16)
            # gathers issue mid-stream so SWDGE descriptor traffic stays
            # light in the ramp-up window; they complete well before the
            # final mult needs them
            gpsimd.wait_ge(tsem, GATHER_AT)
            src = xq_flat.unsqueeze(1)
            for b in range(RB):
                gpsimd.indirect_dma_start(
                    out=tv8[:, b : b + 1],
                    out_offset=None,
                    in_=src,
                    in_offset=bass.IndirectOffsetOnAxis(
                        ap=off_sb[:, b : b + 1], axis=0
                    ),
                ).then_inc(dmaG, 16)

        @block.tensor
        def _(tensor):
            tensor.wait_ge(dmaP, 48)
            lhs8 = w1_sb[:, :].rearrange("p (t x) -> p t x", t=2)[:, :, 0:1]
            for i in range(NTT):
                buf = i % NBUF
                d0, g, _ = xt_tiles[i]
                tensor.wait_ge(dmaT[buf], 16 * (i // NBUF + 1))
                view = data[buf][:, 0 : g * STB].rearrange(
                    "p (g t f) -> p g t f", g=g, t=2
                )
                for j in range(g):
                    d = d0 + j
                    mm = nc.tensor.matmul(
                        rsA[0:1, 0:RTA],
                        lhsT=lhs8,
                        rhs=view[:, j, :, 0:RTA],
                        start=(d == 0),
                        stop=(d == DC - 1),
                        perf_mode=mybir.MatmulPerfMode.DoubleRow,
                    )
                    mm = nc.tensor.matmul(
                        rsB[0:1, 0:RTB],
                        lhsT=lhs8,
                        rhs=view[:, j, :, RTA:RT],
                        start=(d == 0),
                        stop=(d == DC - 1),
                        perf_mode=mybir.MatmulPerfMode.DoubleRow,
                    )
                    if j == g - 1:
                        mm.then_inc(tsem, 1)

        @block.vector
        def _(vector):
            vv = 0
            vector.wait_ge(dmaP, 48)

            def counts(hh):
                nonlocal vv
                for cb in range(RB):
                    nc.vector.tensor_scalar(
                        out=eqscr[:, :],
                        in0=yfb[:, hh * HALF : (hh + 1) * HALF],
                        scalar1=ylf_sb[:, cb : cb + 1],
                        scalar2=None,
                        op0=mybir.AluOpType.is_equal,
                        op1=mybir.AluOpType.add,
                        accum_out=w_half[:, 2 * cb + hh : 2 * cb + hh + 1],
                    ).then_inc(vsem, 1)
                    vv += 1

            vector.wait_ge(bsem, 32)       # yfb labels [0, N/2)
            counts(0)
            vector.wait_ge(bsem, 64)       # yfb labels [N/2, N)
            counts(1)
            vector.wait_ge(vsem, vv)       # flush w_half writes
            nc.vector.tensor_tensor(
                out=w_sb[:, :],
                in0=w_half[:].rearrange("p (b t) -> p b t", t=2)[:, :, 0],
                in1=w_half[:].rearrange("p (b t) -> p b t", t=2)[:, :, 1],
                op=mybir.AluOpType.add,
            ).then_inc(vsem, 1)
            vv += 1
            vector.wait_ge(asem, 1)        # tvf ready (ACT upconvert)
            vector.wait_ge(vsem, vv)       # flush w_sb write
            nc.vector.tensor_tensor(
                out=tprod[:, :], in0=tvf[:, :], in1=w_sb[:, :],
                op=mybir.AluOpType.mult,
            ).then_inc(vsem, 1)
            vv += 1
            # combine the scalar-engine per-chunk rowsum partials per block
            vector.wait_ge(nsem, NBLK * NCH)
            nc.vector.reduce_sum(
                rs_n[:, 0:NBLK],
                act_part[:, :].rearrange("p (b k) -> p b k", k=NCH),
                axis=mybir.AxisListType.X,
            ).then_inc(vsem, 1)
            vv += 1
            assert vv == 2 * RB + 3

        @block.scalar
        def _(scalar):
            V_MULT = 2 * RB + 2
            for i in range(NBLK * NCH):
                buf = i % NNBUF
                scalar.wait_ge(dmaN[buf], 16 * (i // NNBUF + 1))
                nc.scalar.activation(
                    out=act_scr[:, :],
                    in_=ndata[buf][:, :],
                    func=mybir.ActivationFunctionType.Copy,
                    accum_out=act_part[:, i : i + 1],
                ).then_inc(nsem, 1)
            scalar.wait_ge(dmaG, 16 * RB)
            nc.scalar.activation(
                out=tvf[:, :],
                in_=tv8[:, :],
                func=mybir.ActivationFunctionType.Copy,
            ).then_inc(asem, 1)
            scalar.wait_ge(vsem, V_MULT)
            nc.scalar.activation(
                out=logt[:, :],
                in_=tprod[:, :],
                func=mybir.ActivationFunctionType.Ln,
                accum_out=acc[:, 0:1],
            ).then_inc(asem, 1)
            scalar.wait_ge(tsem, NTT)
            nc.scalar.activation(
                out=logr[:, 0:RTA],
                in_=rsA[0:1, 0:RTA],
                func=mybir.ActivationFunctionType.Ln,
                accum_out=acc[0:1, 1:2],
            ).then_inc(asem, 1)
            nc.scalar.activation(
                out=logr[:, 0:RTB],
                in_=rsB[0:1, 0:RTB],
                func=mybir.ActivationFunctionType.Ln,
                accum_out=acc[0:1, 2:3],
            ).then_inc(asem, 1)
            scalar.wait_ge(vsem, 2 * RB + 3)
            nc.scalar.activation(
                out=logn_scr[:, :],
                in_=rs_n[:, 0:NBLK],
                func=mybir.ActivationFunctionType.Ln,
                accum_out=acc[:, 3:4],
            ).then_inc(asem, 1)

    return nc


def _get_nc():
    if "nc" not in _CACHE:
        _CACHE["nc"] = _build_nc()
    return _CACHE["nc"]


def _make_in_maps(output, y):
    import ml_dtypes

    f8 = ml_dtypes.float8_e4m3
    out_f32 = np.asarray(output, dtype=np.float32)
    y64 = np.asarray(y).astype(np.int64)
    yf16 = y64.astype(np.int16).reshape(1, N)
    ones8 = np.ones((P, 32), dtype=f8)
    q8 = (out_f32 * SCALE).astype(f8)

    in_maps = []
    for k in range(NCORES):
        rows = slice(k * RPC, (k + 1) * RPC)
        y_loc = y64[rows]
        # transposed part (rows 0..RT-1): per full tile [128, G, 2, RT],
        # per single tile [128, 1, 2, RT]; element (p, g, t, j) holds
        # column (d0+g)*256 + t*128 + p of local row j
        qk = q8[rows]
        xt = np.ascontiguousarray(qk[:RT].T)           # [C, RT]
        xt4 = xt.reshape(DC, 2, P, RT)                 # [d, t, p, j]
        full = np.ascontiguousarray(
            xt4[: NFT * G].reshape(NFT, G, 2, P, RT).transpose(0, 3, 1, 2, 4)
        )
        tail = np.ascontiguousarray(xt4[NFT * G :].transpose(0, 2, 1, 3))
        xn = qk[RT:]                                   # [RN, C]
        xcore = np.concatenate(
            [full.reshape(-1), tail.reshape(-1), xn.reshape(-1)]
        ).reshape(1, TOTB)

        # (p, b) layout: element (p, b) corresponds to local row b*128 + p
        ylf = np.ascontiguousarray(y_loc.astype(np.float32).reshape(RB, P).T)

        # flat fp8 element offset of (column y[i], local row i) in xcore
        c = y_loc
        i_loc = np.arange(RPC, dtype=np.int64)
        d = c // 256
        t = (c % 256) // 128
        p = c % 128
        in_full = d < NFT * G
        T = np.where(in_full, d // G, 0)
        g = np.where(in_full, d % G, 0)
        base = np.where(
            in_full,
            T * (P * FTB),
            NFT * P * FTB + (d - NFT * G) * (P * STB),
        )
        pitch = np.where(in_full, FTB, STB)
        off_t = base + p * pitch + (g * 2 + t) * RT + i_loc
        off_n = XT_BYTES + (i_loc - RT) * C + c
        off64 = np.where(i_loc < RT, off_t, off_n)
        off = np.ascontiguousarray(off64.astype(np.int32).reshape(RB, P).T)
        in_maps.append(
            {
                "xq": xcore,
                "yf": yf16,
                "ylf": ylf,
                "off": off,
                "w1": ones8,
            }
        )
    return in_maps


def kernel(output, y):
    from concourse.bass_utils import run_bass_kernel_spmd

    output = np.asarray(output)
    y = np.asarray(y)
    assert output.shape == (N, C) and y.shape == (N,)

    in_maps = _make_in_maps(output, y)
    res = run_bass_kernel_spmd(
        _get_nc(), in_maps, core_ids=list(range(NCORES))
    )
    total = 0.0
    for k in range(NCORES):
        o = res.results[k]["out"]
        total += float(o[:, 0].sum(dtype=np.float64)) - float(
            o[0, 1] + o[0, 2] + o[:, 3].sum(dtype=np.float64)
        )
    loss = math.log(N) - total / N
    return np.float32(loss)
